# revision 1
# baseline (speedup 1.0000x reference)
"""Trainium2 Bass kernel for nn_DirectEncodingModel (gnn_message_passing).

Model (reference):
    h = x                                  # [B, 256]
    for l in 0..2:
        gathered = h[:, idx[l]]            # [B, 4, 128]
        z = einsum('bgk,gku->bgu', gathered, W[l]) + b[l]
        h = tanh(z).reshape(B, 256)
    out = h @ W_out + b_out                # [B, 10]

Key transform (host-side, exact): the gather folds into a dense weight
matrix per level:
    Weff[l][d, g*64+u] = sum_{k: idx[l,g,k]==d} W[l,g,k,u]
so each level is h = tanh(h @ Weff[l] + b[l]) — a dense [B,256]@[256,256]
matmul. The device kernel is then a plain 3-layer MLP + output matmul.

Device layout: activations transposed — [feature(partition), batch(free)].
Host pre-transposes x (and casts to bf16); device does bf16 matmuls with
fp32 PSUM accumulation, tanh on the scalar engine (fused per level over
2048-element activations), and writes out^T [10, BS]; host transposes back
and adds b_out (exact, b_out is a constant broadcast).

Sharding: pure data parallelism over the batch axis across 8 cores;
weights replicated.
"""

import numpy as np
import ml_dtypes

import concourse.mybir as mybir
import concourse.bass as bass
import concourse.bacc as bacc
import concourse.tile as tile
from concourse.bass_utils import run_bass_kernel_spmd

BF16 = mybir.dt.bfloat16
F32 = mybir.dt.float32

N_CORES = 8
B, D, L, G, K, U, OUT = 131072, 256, 3, 4, 128, 64, 10
GU = G * U  # 256
BS = B // N_CORES  # 16384 per core

CHUNK = 1024  # batch columns per phase-1 chunk (2 matmul sub-blocks of 512)
NCHUNK = BS // CHUNK

# test-harness hooks (harness never touches these; defaults are production)
TRACE = False
LAST_RESULTS = None


def _build_program(use_bias: bool):
    nc = bacc.Bacc("TRN2", debug=False, target_bir_lowering=False,
                   num_devices=N_CORES)

    xt_d = nc.dram_tensor("xt", [2, 128, BS], BF16, kind="ExternalInput")
    weff_d = nc.dram_tensor("weff", [128, 2 * L, GU], BF16, kind="ExternalInput")
    wout_d = nc.dram_tensor("wout", [128, 2, OUT], BF16, kind="ExternalInput")
    if use_bias:
        bias_d = nc.dram_tensor("bias", [128, 2 * L], F32, kind="ExternalInput")
    outt_d = nc.dram_tensor("outt", [OUT, BS], F32, kind="ExternalOutput")

    Tanh = mybir.ActivationFunctionType.Tanh

    with tile.TileContext(nc) as tc:
        with tc.tile_pool(name="const", bufs=1) as cpool, \
             tc.tile_pool(name="xp", bufs=3) as xpool, \
             tc.tile_pool(name="hp", bufs=2) as hpool, \
             tc.tile_pool(name="h3p", bufs=1) as h3pool:

            # resident weights
            weff_t = cpool.tile([128, 2 * L, GU], BF16)
            nc.sync.dma_start(weff_t[:, :, :], weff_d[:, :, :])
            wout_t = cpool.tile([128, 2, OUT], BF16)
            nc.sync.dma_start(wout_t[:, :, :], wout_d[:, :, :])
            if use_bias:
                bias_t = cpool.tile([128, 2 * L], F32)
                nc.sync.dma_start(bias_t[:, :], bias_d[:, :])

            # all of h3 stays resident in SBUF; consumed by phase 2
            # dims: [part, kt, chunk, ns, 512]
            h3_t = h3pool.tile([128, 2, NCHUNK, 2, 512], BF16)

            # ---- phase 1: the three tanh levels ----
            with tc.tile_pool(name="zp", bufs=2, space="PSUM") as zpool:
                for c in range(NCHUNK):
                    xt_t = xpool.tile([128, 2, CHUNK], BF16, tag="x")
                    for kt in range(2):
                        nc.sync.dma_start(
                            xt_t[:, kt, :],
                            xt_d[kt, :, c * CHUNK:(c + 1) * CHUNK])

                    hprev = None
                    for l in range(L):
                        # z dims: [part, mt, ns, 512] — each [*,mt,ns,:] is
                        # one PSUM bank accumulated over the 2 k-tiles
                        z = zpool.tile([128, 2, 2, 512], F32, tag="z")
                        for mt in range(2):
                            for ns in range(2):
                                for kt in range(2):
                                    if l == 0:
                                        rhs = xt_t[:, kt,
                                                   ns * 512:(ns + 1) * 512]
                                    else:
                                        rhs = hprev[:, kt, ns, :]
                                    nc.tensor.matmul(
                                        z[:, mt, ns, :],
                                        weff_t[:, l * 2 + kt,
                                               mt * 128:(mt + 1) * 128],
                                        rhs,
                                        start=(kt == 0), stop=(kt == 1))
                        if l < L - 1:
                            hcur = hpool.tile([128, 2, 2, 512], BF16,
                                              tag=f"h{l}")
                            if use_bias:
                                for mt in range(2):
                                    nc.scalar.activation(
                                        hcur[:, mt, :, :], z[:, mt, :, :],
                                        Tanh,
                                        bias=bias_t[:, l * 2 + mt:
                                                    l * 2 + mt + 1])
                            else:
                                nc.scalar.activation(
                                    hcur[:, :, :, :], z[:, :, :, :], Tanh)
                            hprev = hcur
                        else:
                            if use_bias:
                                for mt in range(2):
                                    nc.scalar.activation(
                                        h3_t[:, mt, c, :, :], z[:, mt, :, :],
                                        Tanh,
                                        bias=bias_t[:, l * 2 + mt:
                                                    l * 2 + mt + 1])
                            else:
                                nc.scalar.activation(
                                    h3_t[:, :, c, :, :], z[:, :, :, :], Tanh)

            # ---- phase 2: output layer out^T = W_out^T @ h3 ----
            OBLK = 4096  # columns per output DMA
            with tc.tile_pool(name="op", bufs=4, space="PSUM") as opool, \
                 tc.tile_pool(name="obp", bufs=2) as obpool:
                for c2 in range(BS // OBLK):
                    ob = obpool.tile([OUT, OBLK], F32, tag="ob")
                    for q in range(OBLK // 1024):
                        po = opool.tile([OUT, 1024], F32, tag="po")
                        for ns2 in range(2):
                            j = c2 * (OBLK // 512) + q * 2 + ns2
                            for kt in range(2):
                                nc.tensor.matmul(
                                    po[:, ns2 * 512:(ns2 + 1) * 512],
                                    wout_t[:, kt, :],
                                    h3_t[:, kt, j // 2, j % 2, :],
                                    start=(kt == 0), stop=(kt == 1))
                        nc.vector.tensor_copy(ob[:, q * 1024:(q + 1) * 1024],
                                              po[:, :])
                    nc.sync.dma_start(
                        outt_d[:, c2 * OBLK:(c2 + 1) * OBLK], ob[:, :])

    nc.compile()
    return nc


def kernel(x, idx, W, b, W_out, b_out):
    global LAST_RESULTS
    x = np.asarray(x, dtype=np.float32)
    idx = np.asarray(idx, dtype=np.int32)
    W = np.asarray(W, dtype=np.float32)
    b = np.asarray(b, dtype=np.float32)
    W_out = np.asarray(W_out, dtype=np.float32)
    b_out = np.asarray(b_out, dtype=np.float32)

    # fold the gather into dense per-level weights (exact, fp32)
    Weff = np.zeros((L, D, GU), np.float32)
    for l in range(L):
        for g in range(G):
            np.add.at(Weff[l, :, g * U:(g + 1) * U], idx[l, g], W[l, g])

    # device weight layouts (K-tile on partitions)
    weff_dev = np.ascontiguousarray(
        Weff.reshape(L, 2, 128, GU).transpose(2, 0, 1, 3)
        .reshape(128, 2 * L, GU)).astype(ml_dtypes.bfloat16)
    wout_dev = np.ascontiguousarray(
        W_out.reshape(2, 128, OUT).transpose(1, 0, 2)).astype(
        ml_dtypes.bfloat16)

    use_bias = bool(np.any(b != 0.0))
    bias_dev = np.ascontiguousarray(
        b.reshape(L, 2, 128).transpose(2, 0, 1).reshape(128, 2 * L)) \
        if use_bias else None

    nc = _build_program(use_bias)

    in_maps = []
    for c in range(N_CORES):
        xs = x[c * BS:(c + 1) * BS]                       # [BS, 256]
        xt = xs.T.astype(ml_dtypes.bfloat16).reshape(2, 128, BS)
        m = {"xt": np.ascontiguousarray(xt),
             "weff": weff_dev, "wout": wout_dev}
        if use_bias:
            m["bias"] = bias_dev
        in_maps.append(m)

    res = run_bass_kernel_spmd(nc, in_maps, list(range(N_CORES)),
                               trace=TRACE)
    LAST_RESULTS = res

    out = np.empty((B, OUT), np.float32)
    for c in range(N_CORES):
        out[c * BS:(c + 1) * BS] = res.results[c]["outt"].T
    if np.any(b_out != 0.0):
        out += b_out[None, :]
    return out


# revision 4
# speedup vs baseline: 20.9475x; 20.9475x over previous
"""Trainium2 Bass kernel for nn_DirectEncodingModel (gnn_message_passing).

Model (reference):
    h = x                                  # [B, 256]
    for l in 0..2:
        gathered = h[:, idx[l]]            # [B, 4, 128]
        z = einsum('bgk,gku->bgu', gathered, W[l]) + b[l]
        h = tanh(z).reshape(B, 256)
    out = h @ W_out + b_out                # [B, 10]

Key transform (host-side, exact): the gather folds into a dense weight
matrix per level:
    Weff[l][d, g*64+u] = sum_{k: idx[l,g,k]==d} W[l,g,k,u]
so each level is h = tanh(h @ Weff[l] + b[l]) — a dense [B,256]@[256,256]
matmul. The device kernel is then a plain 3-layer MLP + output matmul.

Device layout: activations transposed — [feature(partition), batch(free)].
Host pre-transposes x (and casts to bf16); device does bf16 matmuls with
fp32 PSUM accumulation, tanh on the scalar engine (fused per level over
2048-element activations), and writes out^T [10, BS]; host transposes back
and adds b_out (exact, b_out is a constant broadcast).

Sharding: pure data parallelism over the batch axis across 8 cores;
weights replicated.
"""

import numpy as np
import ml_dtypes

import concourse.mybir as mybir
import concourse.bass as bass
import concourse.bacc as bacc
import concourse.tile as tile
from concourse.bass_utils import run_bass_kernel_spmd

BF16 = mybir.dt.bfloat16
F32 = mybir.dt.float32

N_CORES = 8
B, D, L, G, K, U, OUT = 131072, 256, 3, 4, 128, 64, 10
GU = G * U  # 256
BS = B // N_CORES  # 16384 per core

CHUNK = 1024  # batch columns per phase-1 chunk (2 matmul sub-blocks of 512)
NCHUNK = BS // CHUNK

# test-harness hooks (harness never touches these; defaults are production)
TRACE = False
LAST_RESULTS = None


def _build_program(use_bias: bool, reps: int = 1):
    nc = bacc.Bacc("TRN2", debug=False, target_bir_lowering=False,
                   num_devices=N_CORES)

    xt_d = nc.dram_tensor("xt", [2, 128, BS], BF16, kind="ExternalInput")
    weff_d = nc.dram_tensor("weff", [128, 2 * L, GU], BF16, kind="ExternalInput")
    wout_d = nc.dram_tensor("wout", [128, 2, OUT], BF16, kind="ExternalInput")
    if use_bias:
        bias_d = nc.dram_tensor("bias", [128, 2 * L], F32, kind="ExternalInput")
    outt_d = nc.dram_tensor("outt", [OUT, BS], F32, kind="ExternalOutput")

    Tanh = mybir.ActivationFunctionType.Tanh

    with tile.TileContext(nc) as tc:
        with tc.tile_pool(name="const", bufs=1) as cpool, \
             tc.tile_pool(name="xp", bufs=3) as xpool, \
             tc.tile_pool(name="hp", bufs=2) as hpool, \
             tc.tile_pool(name="h3p", bufs=1) as h3pool:

            # resident weights
            weff_t = cpool.tile([128, 2 * L, GU], BF16)
            nc.sync.dma_start(weff_t[:, :, :], weff_d[:, :, :])
            wout_t = cpool.tile([128, 2, OUT], BF16)
            nc.sync.dma_start(wout_t[:, :, :], wout_d[:, :, :])
            if use_bias:
                bias_t = cpool.tile([128, 2 * L], F32)
                nc.sync.dma_start(bias_t[:, :], bias_d[:, :])

            # all of h3 stays resident in SBUF; consumed by phase 2
            # dims: [part, kt, chunk, ns, 512]
            h3_t = h3pool.tile([128, 2, NCHUNK, 2, 512], BF16)

            for _rep in range(reps):
              # ---- phase 1: the three tanh levels ----
              with tc.tile_pool(name="zp", bufs=2, space="PSUM") as zpool:
                for c in range(NCHUNK):
                    xt_t = xpool.tile([128, 2, CHUNK], BF16, tag="x")
                    for kt in range(2):
                        nc.sync.dma_start(
                            xt_t[:, kt, :],
                            xt_d[kt, :, c * CHUNK:(c + 1) * CHUNK])

                    hprev = None
                    for l in range(L):
                        # z dims: [part, mt, ns, 512] — each [*,mt,ns,:] is
                        # one PSUM bank accumulated over the 2 k-tiles
                        z = zpool.tile([128, 2, 2, 512], F32, tag="z")
                        for mt in range(2):
                            for ns in range(2):
                                for kt in range(2):
                                    if l == 0:
                                        rhs = xt_t[:, kt,
                                                   ns * 512:(ns + 1) * 512]
                                    else:
                                        rhs = hprev[:, kt, ns, :]
                                    nc.tensor.matmul(
                                        z[:, mt, ns, :],
                                        weff_t[:, l * 2 + kt,
                                               mt * 128:(mt + 1) * 128],
                                        rhs,
                                        start=(kt == 0), stop=(kt == 1))
                        if l < L - 1:
                            hcur = hpool.tile([128, 2, 2, 512], BF16,
                                              tag=f"h{l}")
                            if use_bias:
                                for mt in range(2):
                                    nc.scalar.activation(
                                        hcur[:, mt, :, :], z[:, mt, :, :],
                                        Tanh,
                                        bias=bias_t[:, l * 2 + mt:
                                                    l * 2 + mt + 1])
                            else:
                                nc.scalar.activation(
                                    hcur[:, :, :, :], z[:, :, :, :], Tanh)
                            hprev = hcur
                        else:
                            if use_bias:
                                for mt in range(2):
                                    nc.scalar.activation(
                                        h3_t[:, mt, c, :, :], z[:, mt, :, :],
                                        Tanh,
                                        bias=bias_t[:, l * 2 + mt:
                                                    l * 2 + mt + 1])
                            else:
                                nc.scalar.activation(
                                    h3_t[:, :, c, :, :], z[:, :, :, :], Tanh)

              # ---- phase 2: output layer out^T = W_out^T @ h3 ----
              OBLK = 4096  # columns per output DMA
              with tc.tile_pool(name="op", bufs=4, space="PSUM") as opool, \
                   tc.tile_pool(name="obp", bufs=2) as obpool:
                for c2 in range(BS // OBLK):
                    ob = obpool.tile([OUT, OBLK], F32, tag="ob")
                    for q in range(OBLK // 1024):
                        po = opool.tile([OUT, 1024], F32, tag="po")
                        for ns2 in range(2):
                            j = c2 * (OBLK // 512) + q * 2 + ns2
                            for kt in range(2):
                                nc.tensor.matmul(
                                    po[:, ns2 * 512:(ns2 + 1) * 512],
                                    wout_t[:, kt, :],
                                    h3_t[:, kt, j // 2, j % 2, :],
                                    start=(kt == 0), stop=(kt == 1))
                        nc.vector.tensor_copy(ob[:, q * 1024:(q + 1) * 1024],
                                              po[:, :])
                    nc.sync.dma_start(
                        outt_d[:, c2 * OBLK:(c2 + 1) * OBLK], ob[:, :])

    nc.compile()
    return nc


def kernel(x, idx, W, b, W_out, b_out):
    global LAST_RESULTS
    x = np.asarray(x, dtype=np.float32)
    idx = np.asarray(idx, dtype=np.int32)
    W = np.asarray(W, dtype=np.float32)
    b = np.asarray(b, dtype=np.float32)
    W_out = np.asarray(W_out, dtype=np.float32)
    b_out = np.asarray(b_out, dtype=np.float32)

    # fold the gather into dense per-level weights (exact, fp32)
    Weff = np.zeros((L, D, GU), np.float32)
    for l in range(L):
        for g in range(G):
            np.add.at(Weff[l, :, g * U:(g + 1) * U], idx[l, g], W[l, g])

    # device weight layouts (K-tile on partitions)
    weff_dev = np.ascontiguousarray(
        Weff.reshape(L, 2, 128, GU).transpose(2, 0, 1, 3)
        .reshape(128, 2 * L, GU)).astype(ml_dtypes.bfloat16)
    wout_dev = np.ascontiguousarray(
        W_out.reshape(2, 128, OUT).transpose(1, 0, 2)).astype(
        ml_dtypes.bfloat16)

    use_bias = bool(np.any(b != 0.0))
    bias_dev = np.ascontiguousarray(
        b.reshape(L, 2, 128).transpose(2, 0, 1).reshape(128, 2 * L)) \
        if use_bias else None

    nc = _build_program(use_bias)

    in_maps = []
    for c in range(N_CORES):
        xs = x[c * BS:(c + 1) * BS]                       # [BS, 256]
        xt = xs.T.astype(ml_dtypes.bfloat16).reshape(2, 128, BS)
        m = {"xt": np.ascontiguousarray(xt),
             "weff": weff_dev, "wout": wout_dev}
        if use_bias:
            m["bias"] = bias_dev
        in_maps.append(m)

    res = run_bass_kernel_spmd(nc, in_maps, list(range(N_CORES)),
                               trace=TRACE)
    LAST_RESULTS = res

    out = np.empty((B, OUT), np.float32)
    for c in range(N_CORES):
        out[c * BS:(c + 1) * BS] = res.results[c]["outt"].T
    if np.any(b_out != 0.0):
        out += b_out[None, :]
    return out


# revision 8
# speedup vs baseline: 31.7336x; 1.5149x over previous
"""Trainium2 Bass kernel for nn_DirectEncodingModel (gnn_message_passing).

Model (reference):
    h = x                                  # [B, 256]
    for l in 0..2:
        gathered = h[:, idx[l]]            # [B, 4, 128]
        z = einsum('bgk,gku->bgu', gathered, W[l]) + b[l]
        h = tanh(z).reshape(B, 256)
    out = h @ W_out + b_out                # [B, 10]

Key transform (host-side, exact): the gather folds into a dense weight
matrix per level:
    Weff[l][d, g*64+u] = sum_{k: idx[l,g,k]==d} W[l,g,k,u]
so each level is h = tanh(h @ Weff[l] + b[l]) — a dense [B,256]@[256,256]
matmul. The device kernel is then a plain 3-layer MLP + output matmul.

Device layout: activations transposed — [feature(partition), batch(free)].
Host pre-transposes x (and casts to bf16); device does bf16 matmuls with
fp32 PSUM accumulation, tanh on the scalar engine, and writes out^T
[10, BS]; host transposes back and adds b_out (exact: b_out is a constant
broadcast, added in fp32 on host).

Sharding: pure data parallelism over the batch axis across 8 cores;
weights replicated.
"""

import numpy as np
import ml_dtypes

import concourse.mybir as mybir
import concourse.bass as bass
import concourse.bacc as bacc
import concourse.tile as tile
from concourse.bass_utils import run_bass_kernel_spmd

BF16 = mybir.dt.bfloat16
F32 = mybir.dt.float32

N_CORES = 8
B, D, L, G, K, U, OUT = 131072, 256, 3, 4, 128, 64, 10
GU = G * U  # 256
BS = B // N_CORES  # 16384 per core

CHUNK = 512           # batch columns per level-computation (one PSUM slot)
NCHUNK = BS // CHUNK  # 32
XBLK = 2048           # batch columns per x DMA
OBLK = 2048           # batch columns per output DMA

# test-harness hooks (harness never touches these; defaults are production)
TRACE = False
LAST_RESULTS = None


def _build_program(use_bias: bool, reps: int = 1):
    nc = bacc.Bacc("TRN2", debug=False, target_bir_lowering=False,
                   num_devices=N_CORES)

    xt_d = nc.dram_tensor("xt", [2, 128, BS], BF16, kind="ExternalInput")
    weff_d = nc.dram_tensor("weff", [128, 2 * L, GU], BF16, kind="ExternalInput")
    wout_d = nc.dram_tensor("wout", [128, 2, OUT], BF16, kind="ExternalInput")
    if use_bias:
        bias_d = nc.dram_tensor("bias", [128, 2 * L], F32, kind="ExternalInput")
    outt_d = nc.dram_tensor("outt", [OUT, BS], F32, kind="ExternalOutput")

    Tanh = mybir.ActivationFunctionType.Tanh

    with tile.TileContext(nc) as tc:
        with tc.tile_pool(name="const", bufs=1) as cpool, \
             tc.tile_pool(name="xp", bufs=3) as xpool, \
             tc.tile_pool(name="hp", bufs=3) as hpool, \
             tc.tile_pool(name="obp", bufs=2) as obpool, \
             tc.tile_pool(name="zp", bufs=3, space="PSUM") as zpool, \
             tc.tile_pool(name="op", bufs=2, space="PSUM") as opool:

            # resident weights
            weff_t = cpool.tile([128, 2 * L, GU], BF16)
            nc.sync.dma_start(weff_t[:, :, :], weff_d[:, :, :])
            wout_t = cpool.tile([128, 2, OUT], BF16)
            nc.sync.dma_start(wout_t[:, :, :], wout_d[:, :, :])
            if use_bias:
                bias_t = cpool.tile([128, 2 * L], F32)
                nc.sync.dma_start(bias_t[:, :], bias_d[:, :])

            for _rep in range(reps):
                # software-pipelined over chunks: at tick i we emit
                #   out(i-3) | L2(i-2) | L1(i-1) | L0(i)
                # so every instruction in a tick is dep-ready at tick start
                # (its producers ran in earlier ticks) — each engine's
                # in-order stream never head-of-line blocks.
                xts = {}
                hs = [{} for _ in range(L)]  # hs[l][c] = tile holding h_{l+1}(c)
                obs = {}

                def load_x(c):
                    blk = c // (XBLK // CHUNK)
                    if c % (XBLK // CHUNK) == 0:
                        t = xpool.tile([128, 2, XBLK], BF16, tag="x",
                                       name=f"xr{_rep}b{blk}")
                        for kt in range(2):
                            nc.sync.dma_start(
                                t[:, kt, :],
                                xt_d[kt, :, blk * XBLK:(blk + 1) * XBLK])
                        xts[blk] = t

                def level(c, l):
                    z = zpool.tile([128, 2, CHUNK], F32, tag="z",
                                   name=f"zr{_rep}c{c}l{l}")
                    for mt in range(2):
                        for kt in range(2):
                            if l == 0:
                                blk = c // (XBLK // CHUNK)
                                xoff = (c % (XBLK // CHUNK)) * CHUNK
                                rhs = xts[blk][:, kt, xoff:xoff + CHUNK]
                            else:
                                rhs = hs[l - 1][c][:, kt, :]
                            nc.tensor.matmul(
                                z[:, mt, :],
                                weff_t[:, l * 2 + kt, mt * 128:(mt + 1) * 128],
                                rhs,
                                start=(kt == 0), stop=(kt == 1))
                    hcur = hpool.tile([128, 2, CHUNK], BF16, tag=f"h{l}",
                                      name=f"hr{_rep}c{c}l{l}")
                    if use_bias:
                        for mt in range(2):
                            nc.scalar.activation(
                                hcur[:, mt, :], z[:, mt, :], Tanh,
                                bias=bias_t[:, l * 2 + mt:l * 2 + mt + 1])
                    else:
                        nc.scalar.activation(hcur[:, :, :], z[:, :, :], Tanh)
                    hs[l][c] = hcur
                    if l > 0:
                        del hs[l - 1][c]

                def out_layer(c):
                    po = opool.tile([OUT, CHUNK], F32, tag="po",
                                    name=f"por{_rep}c{c}")
                    for kt in range(2):
                        nc.tensor.matmul(po[:, :], wout_t[:, kt, :],
                                         hs[L - 1][c][:, kt, :],
                                         start=(kt == 0), stop=(kt == 1))
                    del hs[L - 1][c]
                    oblk = c // (OBLK // CHUNK)
                    if c % (OBLK // CHUNK) == 0:
                        obs[oblk] = obpool.tile([OUT, OBLK], F32, tag="ob",
                                                name=f"obr{_rep}b{oblk}")
                    ooff = (c % (OBLK // CHUNK)) * CHUNK
                    nc.vector.tensor_copy(obs[oblk][:, ooff:ooff + CHUNK],
                                          po[:, :])
                    if c % (OBLK // CHUNK) == (OBLK // CHUNK) - 1:
                        nc.sync.dma_start(
                            outt_d[:, oblk * OBLK:(oblk + 1) * OBLK],
                            obs[oblk][:, :])
                        del obs[oblk]

                load_x(0)  # prologue prefetch
                for i in range(NCHUNK + L):
                    if i - L >= 0:
                        out_layer(i - L)
                    for l in range(L - 1, -1, -1):
                        c = i - l
                        if 0 <= c < NCHUNK:
                            level(c, l)
                    if i + 1 < NCHUNK:
                        load_x(i + 1)

    nc.compile()
    return nc


def kernel(x, idx, W, b, W_out, b_out):
    global LAST_RESULTS
    x = np.asarray(x, dtype=np.float32)
    idx = np.asarray(idx, dtype=np.int32)
    W = np.asarray(W, dtype=np.float32)
    b = np.asarray(b, dtype=np.float32)
    W_out = np.asarray(W_out, dtype=np.float32)
    b_out = np.asarray(b_out, dtype=np.float32)

    # fold the gather into dense per-level weights (exact, fp32)
    Weff = np.zeros((L, D, GU), np.float32)
    for l in range(L):
        for g in range(G):
            np.add.at(Weff[l, :, g * U:(g + 1) * U], idx[l, g], W[l, g])

    # device weight layouts (K-tile on partitions)
    weff_dev = np.ascontiguousarray(
        Weff.reshape(L, 2, 128, GU).transpose(2, 0, 1, 3)
        .reshape(128, 2 * L, GU)).astype(ml_dtypes.bfloat16)
    wout_dev = np.ascontiguousarray(
        W_out.reshape(2, 128, OUT).transpose(1, 0, 2)).astype(
        ml_dtypes.bfloat16)

    use_bias = bool(np.any(b != 0.0))
    bias_dev = np.ascontiguousarray(
        b.reshape(L, 2, 128).transpose(2, 0, 1).reshape(128, 2 * L)) \
        if use_bias else None

    nc = _build_program(use_bias)

    in_maps = []
    for c in range(N_CORES):
        xs = x[c * BS:(c + 1) * BS]                       # [BS, 256]
        xt = xs.T.astype(ml_dtypes.bfloat16).reshape(2, 128, BS)
        m = {"xt": np.ascontiguousarray(xt),
             "weff": weff_dev, "wout": wout_dev}
        if use_bias:
            m["bias"] = bias_dev
        in_maps.append(m)

    res = run_bass_kernel_spmd(nc, in_maps, list(range(N_CORES)),
                               trace=TRACE)
    LAST_RESULTS = res

    out = np.empty((B, OUT), np.float32)
    for c in range(N_CORES):
        out[c * BS:(c + 1) * BS] = res.results[c]["outt"].T
    if np.any(b_out != 0.0):
        out += b_out[None, :]
    return out


# revision 16
# speedup vs baseline: 116.3739x; 3.6672x over previous
"""Trainium2 Bass kernel for nn_DirectEncodingModel (gnn_message_passing).

Model (reference):
    h = x                                  # [B, 256]
    for l in 0..2:
        gathered = h[:, idx[l]]            # [B, 4, 128]
        z = einsum('bgk,gku->bgu', gathered, W[l]) + b[l]
        h = tanh(z).reshape(B, 256)
    out = h @ W_out + b_out                # [B, 10]

Key transform (host-side, exact): the gather folds into a dense weight
matrix per level:
    Weff[l][d, g*64+u] = sum_{k: idx[l,g,k]==d} W[l,g,k,u]
so each level is h = tanh(h @ Weff[l] + b[l]) — a dense [B,256]@[256,256]
matmul. The device kernel is then a plain 3-layer MLP + output matmul.

Device layout: activations transposed — [feature(partition), batch(free)].
Host pre-transposes x (and casts to bf16); device does bf16 matmuls with
fp32 PSUM accumulation, tanh on the scalar engine, and writes out^T
[10, BS]; host transposes back and adds b_out (exact: b_out is a constant
broadcast, added in fp32 on host).

Sharding: pure data parallelism over the batch axis across 8 cores;
weights replicated.
"""

import numpy as np

import concourse.mybir as mybir
import concourse.bass as bass
import concourse.bacc as bacc
import concourse.tile as tile
from concourse.bass_utils import run_bass_kernel_spmd

F16 = mybir.dt.float16
F32 = mybir.dt.float32

N_CORES = 8
B, D, L, G, K, U, OUT = 131072, 256, 3, 4, 128, 64, 10
GU = G * U  # 256
BS = B // N_CORES  # 16384 per core

CHUNK = 512           # batch columns per level-computation (one PSUM slot)
NCHUNK = BS // CHUNK  # 32
XBLK = 1024           # batch columns per x DMA
OBLK = 1024           # batch columns per output DMA

# test-harness hooks (harness never touches these; defaults are production)
TRACE = False
LAST_RESULTS = None

_PROG_CACHE = {}


def _build_program(use_bias: bool, reps: int = 1):
    nc = bacc.Bacc("TRN2", debug=False, target_bir_lowering=False,
                   num_devices=N_CORES)

    xt_d = nc.dram_tensor("xt", [2, 128, BS], F16, kind="ExternalInput")
    weff_d = nc.dram_tensor("weff", [128, 2 * L, GU], F16, kind="ExternalInput")
    wout_d = nc.dram_tensor("wout", [128, 2, OUT], F16, kind="ExternalInput")
    if use_bias:
        bias_d = nc.dram_tensor("bias", [128, 2 * L], F32, kind="ExternalInput")
    outt_d = nc.dram_tensor("outt", [OUT, BS], F32, kind="ExternalOutput")

    Tanh = mybir.ActivationFunctionType.Tanh

    with tile.TileContext(nc) as tc:
        with tc.tile_pool(name="const", bufs=1) as cpool, \
             tc.tile_pool(name="xp", bufs=4) as xpool, \
             tc.tile_pool(name="hp", bufs=3) as hpool, \
             tc.tile_pool(name="obp", bufs=2) as obpool, \
             tc.tile_pool(name="zp", bufs=3, space="PSUM") as zpool, \
             tc.tile_pool(name="op", bufs=2, space="PSUM") as opool:

            # resident weights; level-0 slice first so the first matmuls
            # don't wait on the full weight load
            weff_t = cpool.tile([128, 2 * L, GU], F16)
            nc.sync.dma_start(weff_t[:, 0:2, :], weff_d[:, 0:2, :])
            nc.sync.dma_start(weff_t[:, 2:2 * L, :], weff_d[:, 2:2 * L, :])
            wout_t = cpool.tile([128, 2, OUT], F16)
            nc.sync.dma_start(wout_t[:, :, :], wout_d[:, :, :])
            if use_bias:
                bias_t = cpool.tile([128, 2 * L], F32)
                nc.sync.dma_start(bias_t[:, :], bias_d[:, :])

            # trigger the ACT tanh table-set load immediately so it overlaps
            # the first x DMA instead of stalling the first real activation
            warm_in = cpool.tile([128, 1], F32)
            warm_out = cpool.tile([128, 1], F16)
            nc.gpsimd.memset(warm_in[:, :], 0.0)
            nc.scalar.activation(warm_out[:, :], warm_in[:, :], Tanh)

            # x DMA blocks: first two at chunk granularity so the pipeline
            # fills fast, the rest at XBLK
            xblocks = [(0, CHUNK), (CHUNK, CHUNK)]
            off = 2 * CHUNK
            while off < BS:
                xblocks.append((off, XBLK))
                off += XBLK
            chunk_block = {}
            for bi, (s, sz) in enumerate(xblocks):
                for c in range(s // CHUNK, (s + sz) // CHUNK):
                    chunk_block[c] = bi

            for _rep in range(reps):
                # software-pipelined over chunks: at tick i we emit
                #   out(i-3) | L2(i-2) | L1(i-1) | L0(i)
                # so every instruction in a tick is dep-ready at tick start
                # (its producers ran in earlier ticks) — each engine's
                # in-order stream never head-of-line blocks.
                xts = {}
                hs = [{} for _ in range(L)]  # hs[l][c] = tile holding h_{l+1}(c)
                obs = {}

                def load_x(c):
                    bi = chunk_block[c]
                    if bi in xts:
                        return
                    s, sz = xblocks[bi]
                    t = xpool.tile([128, 2, sz], F16, tag="x",
                                   name=f"xr{_rep}b{bi}",
                                   padded_shape=[128, 2, XBLK])
                    for kt in range(2):
                        nc.sync.dma_start(t[:, kt, :], xt_d[kt, :, s:s + sz])
                    xts[bi] = t

                def level(c, l):
                    z = zpool.tile([128, 2, CHUNK], F32, tag="z",
                                   name=f"zr{_rep}c{c}l{l}")
                    for mt in range(2):
                        for kt in range(2):
                            if l == 0:
                                bi = chunk_block[c]
                                s, sz = xblocks[bi]
                                xoff = c * CHUNK - s
                                rhs = xts[bi][:, kt, xoff:xoff + CHUNK]
                            else:
                                rhs = hs[l - 1][c][:, kt, :]
                            nc.tensor.matmul(
                                z[:, mt, :],
                                weff_t[:, l * 2 + kt, mt * 128:(mt + 1) * 128],
                                rhs,
                                start=(kt == 0), stop=(kt == 1))
                    hcur = hpool.tile([128, 2, CHUNK], F16, tag=f"h{l}",
                                      name=f"hr{_rep}c{c}l{l}")
                    if use_bias:
                        for mt in range(2):
                            nc.scalar.activation(
                                hcur[:, mt, :], z[:, mt, :], Tanh,
                                bias=bias_t[:, l * 2 + mt:l * 2 + mt + 1])
                    else:
                        nc.scalar.activation(hcur[:, :, :], z[:, :, :], Tanh)
                    hs[l][c] = hcur
                    if l > 0:
                        del hs[l - 1][c]

                def out_layer(c):
                    po = opool.tile([OUT, CHUNK], F32, tag="po",
                                    name=f"por{_rep}c{c}")
                    for kt in range(2):
                        nc.tensor.matmul(po[:, :], wout_t[:, kt, :],
                                         hs[L - 1][c][:, kt, :],
                                         start=(kt == 0), stop=(kt == 1))
                    del hs[L - 1][c]
                    oblk = c // (OBLK // CHUNK)
                    if c % (OBLK // CHUNK) == 0:
                        obs[oblk] = obpool.tile([OUT, OBLK], F32, tag="ob",
                                                name=f"obr{_rep}b{oblk}")
                    ooff = (c % (OBLK // CHUNK)) * CHUNK
                    nc.vector.tensor_copy(obs[oblk][:, ooff:ooff + CHUNK],
                                          po[:, :])
                    if c % (OBLK // CHUNK) == (OBLK // CHUNK) - 1:
                        nc.sync.dma_start(
                            outt_d[:, oblk * OBLK:(oblk + 1) * OBLK],
                            obs[oblk][:, :])
                        del obs[oblk]

                load_x(0)  # prologue prefetch
                load_x(1)
                for i in range(NCHUNK + L):
                    if i - L >= 0:
                        out_layer(i - L)
                    for l in range(L - 1, -1, -1):
                        c = i - l
                        if 0 <= c < NCHUNK:
                            level(c, l)
                    for ahead in (1, 2):
                        if i + ahead < NCHUNK:
                            load_x(i + ahead)

    nc.compile()
    return nc


def kernel(x, idx, W, b, W_out, b_out):
    global LAST_RESULTS
    x = np.asarray(x, dtype=np.float32)
    idx = np.asarray(idx, dtype=np.int32)
    W = np.asarray(W, dtype=np.float32)
    b = np.asarray(b, dtype=np.float32)
    W_out = np.asarray(W_out, dtype=np.float32)
    b_out = np.asarray(b_out, dtype=np.float32)

    # fold the gather into dense per-level weights (exact, fp32)
    Weff = np.zeros((L, D, GU), np.float32)
    for l in range(L):
        for g in range(G):
            np.add.at(Weff[l, :, g * U:(g + 1) * U], idx[l, g], W[l, g])

    # device weight layouts (K-tile on partitions)
    weff_dev = np.ascontiguousarray(
        Weff.reshape(L, 2, 128, GU).transpose(2, 0, 1, 3)
        .reshape(128, 2 * L, GU)).astype(np.float16)
    wout_dev = np.ascontiguousarray(
        W_out.reshape(2, 128, OUT).transpose(1, 0, 2)).astype(
        np.float16)

    use_bias = bool(np.any(b != 0.0))
    bias_dev = np.ascontiguousarray(
        b.reshape(L, 2, 128).transpose(2, 0, 1).reshape(128, 2 * L)) \
        if use_bias else None

    nc = _build_program(use_bias)

    in_maps = []
    for c in range(N_CORES):
        xs = x[c * BS:(c + 1) * BS]                       # [BS, 256]
        xt = xs.T.astype(np.float16).reshape(2, 128, BS)
        m = {"xt": np.ascontiguousarray(xt),
             "weff": weff_dev, "wout": wout_dev}
        if use_bias:
            m["bias"] = bias_dev
        in_maps.append(m)

    res = run_bass_kernel_spmd(nc, in_maps, list(range(N_CORES)),
                               trace=TRACE)
    LAST_RESULTS = res

    out = np.empty((B, OUT), np.float32)
    for c in range(N_CORES):
        out[c * BS:(c + 1) * BS] = res.results[c]["outt"].T
    if np.any(b_out != 0.0):
        out += b_out[None, :]
    return out


# revision 25
# speedup vs baseline: 126.3125x; 1.0854x over previous
"""Trainium2 Bass kernel for nn_DirectEncodingModel (gnn_message_passing).

Model (reference):
    h = x                                  # [B, 256]
    for l in 0..2:
        gathered = h[:, idx[l]]            # [B, 4, 128]
        z = einsum('bgk,gku->bgu', gathered, W[l]) + b[l]
        h = tanh(z).reshape(B, 256)
    out = h @ W_out + b_out                # [B, 10]

Key transform (host-side, exact): the gather folds into a dense weight
matrix per level:
    Weff[l][d, g*64+u] = sum_{k: idx[l,g,k]==d} W[l,g,k,u]
so each level is h = tanh(h @ Weff[l] + b[l]) — a dense [B,256]@[256,256]
matmul. The device kernel is then a plain 3-layer MLP + output matmul.

Device layout: activations transposed — [feature(partition), batch(free)].
Host pre-transposes x (and casts to fp16); device does fp16 matmuls
(1 cycle/row on the PE, same as bf16, 3 more mantissa bits) with fp32
PSUM accumulation, tanh on the scalar engine, and writes out^T [10, BS];
host transposes back and adds b_out (exact: b_out is a constant
broadcast, added in fp32 on host).

The per-chunk schedule is explicitly software-pipelined (skewed emission:
out(i-3) | L2(i-2) | L1(i-1) | L0(i) per tick) so each engine's in-order
stream never blocks on same-tick producers. PSUM budget (8 banks):
3 z-slots of 2 banks (pipeline depth 3) + 2 out-slots of 1 bank.
Steady state: ScalarE (tanh) is the bottleneck at ~101 us busy/core,
TensorE ~96 us, wall ~118 us (cost model), ~0.1 ms measured.

Sharding: pure data parallelism over the batch axis across 8 cores;
weights replicated.
"""

import numpy as np

import concourse.mybir as mybir
import concourse.bass as bass
import concourse.bacc as bacc
import concourse.tile as tile
from concourse.bass_utils import run_bass_kernel_spmd

F16 = mybir.dt.float16
F32 = mybir.dt.float32

N_CORES = 8
B, D, L, G, K, U, OUT = 131072, 256, 3, 4, 128, 64, 10
GU = G * U  # 256
BS = B // N_CORES  # 16384 per core

CHUNK = 512           # batch columns per level-computation (one PSUM slot)
NCHUNK = BS // CHUNK  # 32
XBLK = 1024           # batch columns per x DMA
OBLK = 1024           # batch columns per output DMA

# test-harness hooks (harness never touches these; defaults are production)
TRACE = False
LAST_RESULTS = None

_PROG_CACHE = {}


def _build_program(use_bias: bool, reps: int = 1):
    nc = bacc.Bacc("TRN2", debug=False, target_bir_lowering=False,
                   num_devices=N_CORES)

    # level 0 uses the host-pre-gathered x (one K=128 matmul per group,
    # M=64, pairs run concurrently via PE column tiling); levels 1-2 use
    # the dense folded weights
    xg_d = nc.dram_tensor("xg", [G, 128, BS], F16, kind="ExternalInput")
    w0_d = nc.dram_tensor("w0", [128, G, U], F16, kind="ExternalInput")
    weff_d = nc.dram_tensor("weff", [128, 2 * (L - 1), GU], F16,
                            kind="ExternalInput")
    wout_d = nc.dram_tensor("wout", [128, 2, OUT], F16, kind="ExternalInput")
    if use_bias:
        bias_d = nc.dram_tensor("bias", [128, 2 * L], F32, kind="ExternalInput")
    outt_d = nc.dram_tensor("outt", [OUT, BS], F32, kind="ExternalOutput")

    Tanh = mybir.ActivationFunctionType.Tanh

    with tile.TileContext(nc) as tc:
        with tc.tile_pool(name="const", bufs=1) as cpool, \
             tc.tile_pool(name="xp", bufs=4) as xpool, \
             tc.tile_pool(name="hp", bufs=3) as hpool, \
             tc.tile_pool(name="obp", bufs=2) as obpool, \
             tc.tile_pool(name="zp", bufs=3, space="PSUM") as zpool, \
             tc.tile_pool(name="op", bufs=2, space="PSUM") as opool:

            # resident weights; level-0 weights first so the first matmuls
            # don't wait on the full weight load
            w0_t = cpool.tile([128, G, U], F16)
            nc.sync.dma_start(w0_t[:, :, :], w0_d[:, :, :])
            weff_t = cpool.tile([128, 2 * (L - 1), GU], F16)
            nc.sync.dma_start(weff_t[:, :, :], weff_d[:, :, :])
            wout_t = cpool.tile([128, 2, OUT], F16)
            nc.sync.dma_start(wout_t[:, :, :], wout_d[:, :, :])
            if use_bias:
                bias_t = cpool.tile([128, 2 * L], F32)
                nc.sync.dma_start(bias_t[:, :], bias_d[:, :])

            # trigger the ACT tanh table-set load immediately so it overlaps
            # the first x DMA instead of stalling the first real activation
            warm_in = cpool.tile([128, 1], F32)
            warm_out = cpool.tile([128, 1], F16)
            nc.gpsimd.memset(warm_in[:, :], 0.0)
            nc.scalar.activation(warm_out[:, :], warm_in[:, :], Tanh)

            # x DMA blocks: first two at chunk granularity so the pipeline
            # fills fast, the rest at XBLK
            xblocks = [(0, CHUNK), (CHUNK, CHUNK)]
            off = 2 * CHUNK
            while off < BS:
                xblocks.append((off, XBLK))
                off += XBLK
            chunk_block = {}
            for bi, (s, sz) in enumerate(xblocks):
                for c in range(s // CHUNK, (s + sz) // CHUNK):
                    chunk_block[c] = bi

            for _rep in range(reps):
                # software-pipelined over chunks: at tick i we emit
                #   out(i-3) | L2(i-2) | L1(i-1) | L0(i)
                # so every instruction in a tick is dep-ready at tick start
                # (its producers ran in earlier ticks) — each engine's
                # in-order stream never head-of-line blocks.
                xts = {}
                hs = [{} for _ in range(L)]  # hs[l][c] = tile holding h_{l+1}(c)
                obs = {}

                def load_x(c):
                    bi = chunk_block[c]
                    if bi in xts:
                        return
                    s, sz = xblocks[bi]
                    t = xpool.tile([128, G, sz], F16, tag="x",
                                   name=f"xr{_rep}b{bi}",
                                   padded_shape=[128, G, XBLK])
                    for g in range(G):
                        nc.sync.dma_start(t[:, g, :], xg_d[g, :, s:s + sz])
                    xts[bi] = t

                def level(c, l):
                    z = zpool.tile([128, 2, CHUNK], F32, tag="z",
                                   name=f"zr{_rep}c{c}l{l}")
                    if l == 0:
                        # gathered form: one K=128 matmul per group; the two
                        # M=64 halves of each pair land in distinct PE column
                        # groups (tile_position from base partitions) and run
                        # concurrently
                        bi = chunk_block[c]
                        s, sz = xblocks[bi]
                        xoff = c * CHUNK - s
                        for pair in range(2):
                            for j in range(2):
                                g = 2 * pair + j
                                nc.tensor.matmul(
                                    z[64 * j:64 * (j + 1), pair, :],
                                    w0_t[:, g, :],
                                    xts[bi][:, g, xoff:xoff + CHUNK],
                                    start=True, stop=True)
                    else:
                        for mt in range(2):
                            for kt in range(2):
                                rhs = hs[l - 1][c][:, kt, :]
                                nc.tensor.matmul(
                                    z[:, mt, :],
                                    weff_t[:, (l - 1) * 2 + kt,
                                           mt * 128:(mt + 1) * 128],
                                    rhs,
                                    start=(kt == 0), stop=(kt == 1))
                    hcur = hpool.tile([128, 2, CHUNK], F16, tag=f"h{l}",
                                      name=f"hr{_rep}c{c}l{l}")
                    if use_bias:
                        for mt in range(2):
                            nc.scalar.activation(
                                hcur[:, mt, :], z[:, mt, :], Tanh,
                                bias=bias_t[:, l * 2 + mt:l * 2 + mt + 1])
                    else:
                        nc.scalar.activation(hcur[:, :, :], z[:, :, :], Tanh)
                    hs[l][c] = hcur
                    if l > 0:
                        del hs[l - 1][c]

                def out_layer(c):
                    po = opool.tile([OUT, CHUNK], F32, tag="po",
                                    name=f"por{_rep}c{c}")
                    for kt in range(2):
                        nc.tensor.matmul(po[:, :], wout_t[:, kt, :],
                                         hs[L - 1][c][:, kt, :],
                                         start=(kt == 0), stop=(kt == 1))
                    del hs[L - 1][c]
                    oblk = c // (OBLK // CHUNK)
                    if c % (OBLK // CHUNK) == 0:
                        obs[oblk] = obpool.tile([OUT, OBLK], F32, tag="ob",
                                                name=f"obr{_rep}b{oblk}")
                    ooff = (c % (OBLK // CHUNK)) * CHUNK
                    nc.vector.tensor_copy(obs[oblk][:, ooff:ooff + CHUNK],
                                          po[:, :])
                    if c % (OBLK // CHUNK) == (OBLK // CHUNK) - 1:
                        nc.sync.dma_start(
                            outt_d[:, oblk * OBLK:(oblk + 1) * OBLK],
                            obs[oblk][:, :])
                        del obs[oblk]

                load_x(0)  # prologue prefetch
                load_x(1)
                for i in range(NCHUNK + L):
                    if i - L >= 0:
                        out_layer(i - L)
                    for l in range(L - 1, -1, -1):
                        c = i - l
                        if 0 <= c < NCHUNK:
                            level(c, l)
                    for ahead in (1, 2):
                        if i + ahead < NCHUNK:
                            load_x(i + ahead)

    nc.compile()
    return nc


def _prepare_in_maps(x, idx, W, b, W_out):
    """Host-side prep: weight folding, layouts, shard + transpose + cast."""
    # fold the gather into dense per-level weights for levels 1..L-1
    # (exact, fp32); level 0 keeps raw per-group weights and uses
    # host-pre-gathered x instead
    Weff = np.zeros((L - 1, D, GU), np.float32)
    for l in range(1, L):
        for g in range(G):
            np.add.at(Weff[l - 1, :, g * U:(g + 1) * U], idx[l, g], W[l, g])

    # device weight layouts (K-tile on partitions)
    weff_dev = np.ascontiguousarray(
        Weff.reshape(L - 1, 2, 128, GU).transpose(2, 0, 1, 3)
        .reshape(128, 2 * (L - 1), GU)).astype(np.float16)
    w0_dev = np.ascontiguousarray(
        W[0].transpose(1, 0, 2)).astype(np.float16)       # [128, G, U]
    wout_dev = np.ascontiguousarray(
        W_out.reshape(2, 128, OUT).transpose(1, 0, 2)).astype(
        np.float16)
    idx0 = idx[0].reshape(-1)                             # [G*K]

    use_bias = bool(np.any(b != 0.0))
    bias_dev = np.ascontiguousarray(
        b.reshape(L, 2, 128).transpose(2, 0, 1).reshape(128, 2 * L)) \
        if use_bias else None

    in_maps = []
    for c in range(N_CORES):
        xs = x[c * BS:(c + 1) * BS]                       # [BS, 256]
        xt = xs.T.astype(np.float16)                      # [256, BS] contig
        xg = xt[idx0].reshape(G, 128, BS)                 # gathered, [G,128,BS]
        m = {"xg": np.ascontiguousarray(xg),
             "w0": w0_dev, "weff": weff_dev, "wout": wout_dev}
        if use_bias:
            m["bias"] = bias_dev
        in_maps.append(m)
    return in_maps, use_bias


def kernel(x, idx, W, b, W_out, b_out):
    global LAST_RESULTS
    x = np.asarray(x, dtype=np.float32)
    idx = np.asarray(idx, dtype=np.int32)
    W = np.asarray(W, dtype=np.float32)
    b = np.asarray(b, dtype=np.float32)
    W_out = np.asarray(W_out, dtype=np.float32)
    b_out = np.asarray(b_out, dtype=np.float32)

    in_maps, use_bias = _prepare_in_maps(x, idx, W, b, W_out)

    nc = _PROG_CACHE.get(use_bias)
    if nc is None:
        nc = _PROG_CACHE[use_bias] = _build_program(use_bias)

    res = run_bass_kernel_spmd(nc, in_maps, list(range(N_CORES)),
                               trace=TRACE)
    LAST_RESULTS = res

    out = np.empty((B, OUT), np.float32)
    for c in range(N_CORES):
        out[c * BS:(c + 1) * BS] = res.results[c]["outt"].T
    if np.any(b_out != 0.0):
        out += b_out[None, :]
    return out
